# revision 12
# baseline (speedup 1.0000x reference)
"""Causal self-attention (B=4, L=2048, D=1024, H=16) on 8 Trainium2 NeuronCores.

Sharding: core c -> (batch b = c//2, head-group g = c%2 of 8 heads).
Each core computes qkv projection for its 8 heads, causal attention, and a
partial out-projection (its head-group's rows of W_out). The host sums the
two partials per batch and adds biases (exact: out-proj is linear and the
v-bias passes through softmax-weighted averaging).

All matmuls run as float32r (fp32 data, reduced-precision PE mode, ~1.5e-4
rel err, 4x the throughput of full fp32).

Attention layout (transpose-free):
  qT, kT   [64d x L]  per head (2 heads stacked per 128 partitions)
  S^T tile [128k x 512q] = kT_tile.T @ qT_block   (PE, K=64)
  expS     = exp(S^T)  (ACT, PSUM->SBUF), causal-masked on diagonal tiles
  O^T,sums [65 x 512q] += [V_tile | ones].T-form @ expS  (PE, K=128)
  O^T_norm = O^T * broadcast(1/sums)  -> directly the lhsT of out-proj
  Y tile   [128l x 512e] = sum_pairs O^T_pair.T @ Wo_pair
"""

import os
from contextlib import ExitStack

import numpy as np

os.environ.setdefault("JAX_PLATFORMS", "")

import concourse.bass as bass
import concourse.mybir as mybir
import concourse.tile as tile
from concourse import bacc, bass_utils

F32 = mybir.dt.float32
F32R = mybir.dt.float32r
AF = mybir.ActivationFunctionType

B, L, D, H = 4, 2048, 1024, 16
DK = D // H            # 64
G = 2                  # head groups (tensor parallel)
HPG = H // G           # 8 heads per group
GW = HPG * DK          # 512 columns per group
P = 128
CO = D // P            # 8 contraction tiles for projections
LT = L // P            # 16 l-tiles / k-tiles
QW = 512               # q-block width
QB = L // QW           # 4 q-blocks
NPAIR = HPG // 2       # 4 head-pairs per group (2 heads per 128 partitions)

_NC_CACHE: dict = {}


def build_nc(with_qk_bias: bool):
    nc = bacc.Bacc("TRN2", target_bir_lowering=False, debug=False, num_devices=8)

    xt = nc.dram_tensor("xt", [D, L], F32, kind="ExternalInput").ap()
    wq = nc.dram_tensor("wq", [D, GW], F32, kind="ExternalInput").ap()
    wk = nc.dram_tensor("wk", [D, GW], F32, kind="ExternalInput").ap()
    wv = nc.dram_tensor("wv", [D, GW], F32, kind="ExternalInput").ap()
    wo = nc.dram_tensor("wo", [GW, D], F32, kind="ExternalInput").ap()
    mband = nc.dram_tensor("mband", [P, 7 * P], F32, kind="ExternalInput").ap()
    if with_qk_bias:
        bq = nc.dram_tensor("bq", [P, NPAIR], F32, kind="ExternalInput").ap()
        bk = nc.dram_tensor("bk", [P, NPAIR], F32, kind="ExternalInput").ap()
    y = nc.dram_tensor("y", [L, D], F32, kind="ExternalOutput").ap()

    xt_r = xt.rearrange("(co p) l -> co p l", p=P)
    wq_r = wq.rearrange("(co p) c -> co p c", p=P)
    wk_r = wk.rearrange("(co p) c -> co p c", p=P)
    wv_r = wv.rearrange("(co p) c -> co p c", p=P)
    wo_r = wo.rearrange("(pr p) e -> pr p e", p=P)
    y_r = y.rearrange("(lt p) e -> lt p e", p=P)

    def mm(out, lhsT, rhs, start, stop):
        nc.tensor.matmul(out, lhsT, rhs, start=start, stop=stop)

    with tile.TileContext(nc) as tc, ExitStack() as ctx:
        constp = ctx.enter_context(tc.tile_pool(name="const", bufs=1))
        # causal band mask: mband[k, i] = 1.0 iff i - 384 >= k
        mband_sb = constp.tile([P, 7 * P], F32)
        nc.sync.dma_start(mband_sb[:], mband)
        # all-ones column (last col of the band mask), f32r-typed
        ones_sb = constp.tile([P, 1], F32R)
        nc.sync.dma_start(ones_sb[:], mband[:, 7 * P - 1:7 * P].bitcast(F32R))
        if with_qk_bias:
            bq_sb = constp.tile([P, NPAIR], F32)
            bk_sb = constp.tile([P, NPAIR], F32)
            nc.sync.dma_start(bq_sb[:], bq)
            nc.sync.dma_start(bk_sb[:], bk)

        qkp = ctx.enter_context(tc.tile_pool(name="qk", bufs=1))
        qT = qkp.tile([P, NPAIR, L], F32R)   # [d-in-pair, pair, l]
        kT = qkp.tile([P, NPAIR, L], F32R)
        vp = ctx.enter_context(tc.tile_pool(name="v", bufs=1))
        vext = vp.tile([P, LT, HPG, DK + 1], F32R)  # [l-in-tile, ltile, head, d|ones]
        otp = ctx.enter_context(tc.tile_pool(name="ot", bufs=2))
        wop = ctx.enter_context(tc.tile_pool(name="wo", bufs=1))

        # ---------------- phase 1: qkv projection ----------------
        # x processed in L-halves to fit SBUF (~160 KB/partition budget)
        LH = L // 2
        with tc.tile_pool(name="xt", bufs=1) as xtp, \
             tc.tile_pool(name="w", bufs=1) as wp, \
             tc.tile_pool(name="psq", bufs=4, space="PSUM") as psq:
            for lh in range(2):
                lbase = lh * LH
                xt_sb = xtp.tile([P, CO, LH], F32R, tag="xt")
                for co in range(CO):
                    nc.sync.dma_start(xt_sb[:, co], xt_r[co, :, lbase:lbase + LH].bitcast(F32R))

                for w_dram, dest, bias_sb in (
                    (wq_r, qT, "bq"), (wk_r, kT, "bk")):
                    w_sb = wp.tile([P, CO, GW], F32R, tag="w")
                    for co in range(CO):
                        nc.sync.dma_start(w_sb[:, co], w_dram[co].bitcast(F32R))
                    for pair in range(NPAIR):
                        for lc in range(LH // QW):
                            pt = psq.tile([P, QW], F32, tag="pq")
                            for co in range(CO):
                                mm(pt[:],
                                   w_sb[:, co, pair * P:(pair + 1) * P],
                                   xt_sb[:, co, lc * QW:(lc + 1) * QW],
                                   start=co == 0, stop=co == CO - 1)
                            dslice = dest[:, pair,
                                          lbase + lc * QW:lbase + (lc + 1) * QW]
                            if with_qk_bias:
                                bt = bq_sb if bias_sb == "bq" else bk_sb
                                nc.vector.tensor_scalar_add(
                                    dslice, pt[:], bt[:, pair:pair + 1])
                            else:
                                nc.scalar.copy(dslice, pt[:])

                w_sb = wp.tile([P, CO, GW], F32R, tag="w")
                for co in range(CO):
                    nc.sync.dma_start(w_sb[:, co], wv_r[co].bitcast(F32R))
                for lt in range(lh * LT // 2, (lh + 1) * LT // 2):
                    pv = psq.tile([P, GW], F32, tag="pv")
                    for co in range(CO):
                        mm(pv[:],
                           xt_sb[:, co, lt * P - lbase:(lt + 1) * P - lbase],
                           w_sb[:, co],
                           start=co == 0, stop=co == CO - 1)
                    nc.vector.tensor_copy(
                        vext[:, lt, :, 0:DK],
                        pv[:].rearrange("p (h d) -> p h d", h=HPG))
                    nc.vector.tensor_copy(
                        vext[:, lt, :, DK:DK + 1],
                        ones_sb[:, :, None].to_broadcast((P, HPG, 1)))

        # allocated after phase-1 pools close so they reuse the freed space
        wo_sb = wop.tile([P, NPAIR, D], F32R)
        for pair in range(NPAIR):
            nc.sync.dma_start(wo_sb[:, pair], wo_r[pair].bitcast(F32R))

        # ------------- phase 2+3: attention + out-projection -------------
        with tc.tile_pool(name="es", bufs=2) as esp, \
             tc.tile_pool(name="rc", bufs=2) as rcp, \
             tc.tile_pool(name="yb", bufs=2) as ybp, \
             tc.tile_pool(name="ps", bufs=2, space="PSUM") as pss, \
             tc.tile_pool(name="po", bufs=2, space="PSUM") as pso, \
             tc.tile_pool(name="py", bufs=2, space="PSUM") as psy:
            for qb in range(QB):
                nj = 4 * qb + 4          # number of valid k-tiles
                # per-q-block O^T accumulator [d-in-pair, pair, q] — recycled
                oT = otp.tile([P, NPAIR, QW], F32R, tag="ot")
                for h in range(HPG):
                    pair, hb = h // 2, (h % 2) * DK
                    qs = qT[hb:hb + DK, pair, qb * QW:(qb + 1) * QW]
                    po = pso.tile([DK + 1, QW], F32, tag="po")
                    for jj in range(0, nj, 2):
                        ps2 = pss.tile([P, 2 * QW], F32, tag="ps")
                        es2 = esp.tile([P, 2 * QW], F32R, tag="es")
                        for u in range(2):
                            j = jj + u
                            mm(ps2[:, u * QW:(u + 1) * QW],
                               kT[hb:hb + DK, pair, j * P:(j + 1) * P],
                               qs, start=True, stop=True)
                        nc.scalar.activation(es2[:], ps2[:], AF.Exp)
                        for u in range(2):
                            j = jj + u
                            o = j - 4 * qb
                            if o >= 0:  # diagonal tile: causal band mask
                                base = u * QW
                                w = (o + 1) * P
                                dj = es2[:, base:base + w]
                                nc.vector.tensor_mul(
                                    dj, dj, mband_sb[:, 3 * P - o * P:3 * P - o * P + w])
                            mm(po[:], vext[:, j, h, :],
                               es2[:, u * QW:(u + 1) * QW],
                               start=j == 0, stop=j == nj - 1)
                    rc = rcp.tile([1, QW], F32, tag="rc")
                    rcb = rcp.tile([DK, QW], F32, tag="rcb")
                    nc.vector.reciprocal(rc[:], po[DK:DK + 1, :])
                    nc.gpsimd.partition_broadcast(rcb[:], rc[:])
                    nc.vector.tensor_mul(
                        oT[hb:hb + DK, pair, :], po[0:DK, :], rcb[:])
                # out-projection for the l-tiles of this q-block
                for lt in range(4 * qb, 4 * qb + 4):
                    lo = (lt - 4 * qb) * P
                    yb = ybp.tile([P, D], F32, tag="yb")
                    for eh in range(2):
                        py = psy.tile([P, QW], F32, tag="py")
                        for pair in range(NPAIR):
                            mm(py[:],
                               oT[:, pair, lo:lo + P],
                               wo_sb[:, pair, eh * QW:(eh + 1) * QW],
                               start=pair == 0, stop=pair == NPAIR - 1)
                        nc.vector.tensor_copy(yb[:, eh * QW:(eh + 1) * QW], py[:])
                    nc.sync.dma_start(y_r[lt], yb[:])

    nc.compile()
    return nc


def _prep_inputs(x, W_qkv, b_qkv, W_out):
    """Per-core input maps. Core c -> batch c//2, head-group c%2."""
    x = np.ascontiguousarray(np.asarray(x, dtype=np.float32))
    W_qkv = np.asarray(W_qkv, dtype=np.float32)
    b_qkv = np.asarray(b_qkv, dtype=np.float32)
    W_out = np.asarray(W_out, dtype=np.float32)

    scale = 1.0 / np.sqrt(DK)
    mband = (np.arange(7 * P)[None, :] - 3 * P
             >= np.arange(P)[:, None]).astype(np.float32)

    with_qk_bias = bool(np.any(b_qkv[:2 * D]))
    xts = [np.ascontiguousarray(x[b].T) for b in range(B)]
    in_maps = []
    for c in range(8):
        b, g = c // 2, c % 2
        sl = slice(g * GW, (g + 1) * GW)
        m = {
            "xt": xts[b],
            "wq": np.ascontiguousarray(W_qkv[:, g * GW:(g + 1) * GW]) * scale,
            "wk": np.ascontiguousarray(W_qkv[:, D + g * GW:D + (g + 1) * GW]),
            "wv": np.ascontiguousarray(W_qkv[:, 2 * D + g * GW:2 * D + (g + 1) * GW]),
            "wo": np.ascontiguousarray(W_out[sl, :]),
            "mband": mband,
        }
        if with_qk_bias:
            m["bq"] = np.ascontiguousarray(
                b_qkv[g * GW:(g + 1) * GW].reshape(NPAIR, P).T) * scale
            m["bk"] = np.ascontiguousarray(
                b_qkv[D + g * GW:D + (g + 1) * GW].reshape(NPAIR, P).T)
        in_maps.append(m)
    return in_maps, with_qk_bias


def kernel(x, W_qkv, b_qkv, W_out, b_out):
    in_maps, with_qk_bias = _prep_inputs(x, W_qkv, b_qkv, W_out)

    key = ("nc", with_qk_bias)
    if key not in _NC_CACHE:
        _NC_CACHE[key] = build_nc(with_qk_bias)
    nc = _NC_CACHE[key]

    res = bass_utils.run_bass_kernel_spmd(nc, in_maps, core_ids=list(range(8)))
    parts = [r["y"] for r in res.results]

    b_qkv = np.asarray(b_qkv, dtype=np.float32)
    W_out_np = np.asarray(W_out, dtype=np.float32)
    # v-bias passes through attention (rows of attn sum to 1) and out-proj is
    # linear: contribution = b_v @ W_out; b_out adds directly.
    corr = (b_qkv[2 * D:3 * D] @ W_out_np
            + np.asarray(b_out, dtype=np.float32)).astype(np.float32)

    out = np.empty((B, L, D), dtype=np.float32)
    for b in range(B):
        out[b] = parts[2 * b] + parts[2 * b + 1] + corr
    return out


# revision 13
# speedup vs baseline: 8.0942x; 8.0942x over previous
"""Causal self-attention (B=4, L=2048, D=1024, H=16) on 8 Trainium2 NeuronCores.

Sharding: core c -> (batch b = c//2, head-group g = c%2 of 8 heads).
Each core computes qkv projection for its 8 heads, causal attention, and a
partial out-projection (its head-group's rows of W_out). The host sums the
two partials per batch and adds biases (exact: out-proj is linear and the
v-bias passes through softmax-weighted averaging).

All matmuls run as float32r (fp32 data, reduced-precision PE mode, ~1.5e-4
rel err, 4x the throughput of full fp32).

Attention layout (transpose-free):
  qT, kT   [64d x L]  per head (2 heads stacked per 128 partitions)
  S^T tile [128k x 512q] = kT_tile.T @ qT_block   (PE, K=64)
  expS     = exp(S^T)  (ACT, PSUM->SBUF), causal-masked on diagonal tiles
  O^T,sums [65 x 512q] += [V_tile | ones].T-form @ expS  (PE, K=128)
  O^T_norm = O^T * broadcast(1/sums)  -> directly the lhsT of out-proj
  Y tile   [128l x 512e] = sum_pairs O^T_pair.T @ Wo_pair
"""

import os
from contextlib import ExitStack

import numpy as np

os.environ.setdefault("JAX_PLATFORMS", "")

import concourse.bass as bass
import concourse.mybir as mybir
import concourse.tile as tile
from concourse import bacc, bass_utils

F32 = mybir.dt.float32
F32R = mybir.dt.float32r
AF = mybir.ActivationFunctionType

B, L, D, H = 4, 2048, 1024, 16
DK = D // H            # 64
G = 2                  # head groups (tensor parallel)
HPG = H // G           # 8 heads per group
GW = HPG * DK          # 512 columns per group
P = 128
CO = D // P            # 8 contraction tiles for projections
LT = L // P            # 16 l-tiles / k-tiles
QW = 512               # q-block width
QB = L // QW           # 4 q-blocks
NPAIR = HPG // 2       # 4 head-pairs per group (2 heads per 128 partitions)

_NC_CACHE: dict = {}


def build_nc(with_qk_bias: bool, repeat: int = 1):
    nc = bacc.Bacc("TRN2", target_bir_lowering=False, debug=False, num_devices=8)

    xt = nc.dram_tensor("xt", [D, L], F32, kind="ExternalInput").ap()
    wq = nc.dram_tensor("wq", [D, GW], F32, kind="ExternalInput").ap()
    wk = nc.dram_tensor("wk", [D, GW], F32, kind="ExternalInput").ap()
    wv = nc.dram_tensor("wv", [D, GW], F32, kind="ExternalInput").ap()
    wo = nc.dram_tensor("wo", [GW, D], F32, kind="ExternalInput").ap()
    mband = nc.dram_tensor("mband", [P, 7 * P], F32, kind="ExternalInput").ap()
    if with_qk_bias:
        bq = nc.dram_tensor("bq", [P, NPAIR], F32, kind="ExternalInput").ap()
        bk = nc.dram_tensor("bk", [P, NPAIR], F32, kind="ExternalInput").ap()
    y = nc.dram_tensor("y", [L, D], F32, kind="ExternalOutput").ap()

    xt_r = xt.rearrange("(co p) l -> co p l", p=P)
    wq_r = wq.rearrange("(co p) c -> co p c", p=P)
    wk_r = wk.rearrange("(co p) c -> co p c", p=P)
    wv_r = wv.rearrange("(co p) c -> co p c", p=P)
    wo_r = wo.rearrange("(pr p) e -> pr p e", p=P)
    y_r = y.rearrange("(lt p) e -> lt p e", p=P)

    def mm(out, lhsT, rhs, start, stop):
        nc.tensor.matmul(out, lhsT, rhs, start=start, stop=stop)

    with tile.TileContext(nc) as tc, ExitStack() as ctx:
        constp = ctx.enter_context(tc.tile_pool(name="const", bufs=1))
        # causal band mask: mband[k, i] = 1.0 iff i - 384 >= k
        mband_sb = constp.tile([P, 7 * P], F32)
        nc.sync.dma_start(mband_sb[:], mband)
        # all-ones column (last col of the band mask), f32r-typed
        ones_sb = constp.tile([P, 1], F32R)
        nc.sync.dma_start(ones_sb[:], mband[:, 7 * P - 1:7 * P].bitcast(F32R))
        if with_qk_bias:
            bq_sb = constp.tile([P, NPAIR], F32)
            bk_sb = constp.tile([P, NPAIR], F32)
            nc.sync.dma_start(bq_sb[:], bq)
            nc.sync.dma_start(bk_sb[:], bk)

        qkp = ctx.enter_context(tc.tile_pool(name="qk", bufs=1))
        qT = qkp.tile([P, NPAIR, L], F32R)   # [d-in-pair, pair, l]
        kT = qkp.tile([P, NPAIR, L], F32R)
        vp = ctx.enter_context(tc.tile_pool(name="v", bufs=1))
        vext = vp.tile([P, LT, HPG, DK + 1], F32R)  # [l-in-tile, ltile, head, d|ones]
        otp = ctx.enter_context(tc.tile_pool(name="ot", bufs=2))
        wop = ctx.enter_context(tc.tile_pool(name="wo", bufs=1))

        for _rep in range(repeat):
            _kernel_body(nc, tc, mm, with_qk_bias, locals())

    nc.compile()
    return nc


def _kernel_body(nc, tc, mm, with_qk_bias, env):
    qT, kT, vext, otp, wop = (env["qT"], env["kT"], env["vext"],
                              env["otp"], env["wop"])
    constp = env["constp"]
    mband_sb, ones_sb = env["mband_sb"], env["ones_sb"]
    xt_r, wq_r, wk_r, wv_r, wo_r, y_r = (env["xt_r"], env["wq_r"], env["wk_r"],
                                         env["wv_r"], env["wo_r"], env["y_r"])
    bq_sb = env.get("bq_sb")
    bk_sb = env.get("bk_sb")
    if True:
        # ---------------- phase 1: qkv projection ----------------
        # x processed in L-halves to fit SBUF (~160 KB/partition budget)
        LH = L // 2
        with tc.tile_pool(name="xt", bufs=1) as xtp, \
             tc.tile_pool(name="w", bufs=1) as wp, \
             tc.tile_pool(name="psq", bufs=4, space="PSUM") as psq:
            for lh in range(2):
                lbase = lh * LH
                xt_sb = xtp.tile([P, CO, LH], F32R, tag="xt")
                for co in range(CO):
                    nc.sync.dma_start(xt_sb[:, co], xt_r[co, :, lbase:lbase + LH].bitcast(F32R))

                for w_dram, dest, bias_sb in (
                    (wq_r, qT, "bq"), (wk_r, kT, "bk")):
                    w_sb = wp.tile([P, CO, GW], F32R, tag="w")
                    for co in range(CO):
                        nc.sync.dma_start(w_sb[:, co], w_dram[co].bitcast(F32R))
                    for pair in range(NPAIR):
                        for lc in range(LH // QW):
                            pt = psq.tile([P, QW], F32, tag="pq")
                            for co in range(CO):
                                mm(pt[:],
                                   w_sb[:, co, pair * P:(pair + 1) * P],
                                   xt_sb[:, co, lc * QW:(lc + 1) * QW],
                                   start=co == 0, stop=co == CO - 1)
                            dslice = dest[:, pair,
                                          lbase + lc * QW:lbase + (lc + 1) * QW]
                            if with_qk_bias:
                                bt = bq_sb if bias_sb == "bq" else bk_sb
                                nc.vector.tensor_scalar_add(
                                    dslice, pt[:], bt[:, pair:pair + 1])
                            else:
                                nc.scalar.copy(dslice, pt[:])

                w_sb = wp.tile([P, CO, GW], F32R, tag="w")
                for co in range(CO):
                    nc.sync.dma_start(w_sb[:, co], wv_r[co].bitcast(F32R))
                for lt in range(lh * LT // 2, (lh + 1) * LT // 2):
                    pv = psq.tile([P, GW], F32, tag="pv")
                    for co in range(CO):
                        mm(pv[:],
                           xt_sb[:, co, lt * P - lbase:(lt + 1) * P - lbase],
                           w_sb[:, co],
                           start=co == 0, stop=co == CO - 1)
                    nc.vector.tensor_copy(
                        vext[:, lt, :, 0:DK],
                        pv[:].rearrange("p (h d) -> p h d", h=HPG))
                    nc.vector.tensor_copy(
                        vext[:, lt, :, DK:DK + 1],
                        ones_sb[:, :, None].to_broadcast((P, HPG, 1)))

        # allocated after phase-1 pools close so they reuse the freed space
        wo_sb = wop.tile([P, NPAIR, D], F32R)
        for pair in range(NPAIR):
            nc.sync.dma_start(wo_sb[:, pair], wo_r[pair].bitcast(F32R))

        # ------------- phase 2+3: attention + out-projection -------------
        with tc.tile_pool(name="es", bufs=2) as esp, \
             tc.tile_pool(name="rc", bufs=2) as rcp, \
             tc.tile_pool(name="yb", bufs=2) as ybp, \
             tc.tile_pool(name="ps", bufs=2, space="PSUM") as pss, \
             tc.tile_pool(name="po", bufs=2, space="PSUM") as pso, \
             tc.tile_pool(name="py", bufs=2, space="PSUM") as psy:
            for qb in range(QB):
                nj = 4 * qb + 4          # number of valid k-tiles
                # per-q-block O^T accumulator [d-in-pair, pair, q] — recycled
                oT = otp.tile([P, NPAIR, QW], F32R, tag="ot")
                for h in range(HPG):
                    pair, hb = h // 2, (h % 2) * DK
                    qs = qT[hb:hb + DK, pair, qb * QW:(qb + 1) * QW]
                    po = pso.tile([DK + 1, QW], F32, tag="po")
                    for jj in range(0, nj, 2):
                        ps2 = pss.tile([P, 2 * QW], F32, tag="ps")
                        es2 = esp.tile([P, 2 * QW], F32R, tag="es")
                        for u in range(2):
                            j = jj + u
                            mm(ps2[:, u * QW:(u + 1) * QW],
                               kT[hb:hb + DK, pair, j * P:(j + 1) * P],
                               qs, start=True, stop=True)
                        nc.scalar.activation(es2[:], ps2[:], AF.Exp)
                        for u in range(2):
                            j = jj + u
                            o = j - 4 * qb
                            if o >= 0:  # diagonal tile: causal band mask
                                base = u * QW
                                w = (o + 1) * P
                                dj = es2[:, base:base + w]
                                nc.vector.tensor_mul(
                                    dj, dj, mband_sb[:, 3 * P - o * P:3 * P - o * P + w])
                            mm(po[:], vext[:, j, h, :],
                               es2[:, u * QW:(u + 1) * QW],
                               start=j == 0, stop=j == nj - 1)
                    rc = rcp.tile([1, QW], F32, tag="rc")
                    rcb = rcp.tile([DK, QW], F32, tag="rcb")
                    nc.vector.reciprocal(rc[:], po[DK:DK + 1, :])
                    nc.gpsimd.partition_broadcast(rcb[:], rc[:])
                    nc.vector.tensor_mul(
                        oT[hb:hb + DK, pair, :], po[0:DK, :], rcb[:])
                # out-projection for the l-tiles of this q-block
                for lt in range(4 * qb, 4 * qb + 4):
                    lo = (lt - 4 * qb) * P
                    yb = ybp.tile([P, D], F32, tag="yb")
                    for eh in range(2):
                        py = psy.tile([P, QW], F32, tag="py")
                        for pair in range(NPAIR):
                            mm(py[:],
                               oT[:, pair, lo:lo + P],
                               wo_sb[:, pair, eh * QW:(eh + 1) * QW],
                               start=pair == 0, stop=pair == NPAIR - 1)
                        nc.vector.tensor_copy(yb[:, eh * QW:(eh + 1) * QW], py[:])
                    nc.sync.dma_start(y_r[lt], yb[:])


def _prep_inputs(x, W_qkv, b_qkv, W_out):
    """Per-core input maps. Core c -> batch c//2, head-group c%2."""
    x = np.ascontiguousarray(np.asarray(x, dtype=np.float32))
    W_qkv = np.asarray(W_qkv, dtype=np.float32)
    b_qkv = np.asarray(b_qkv, dtype=np.float32)
    W_out = np.asarray(W_out, dtype=np.float32)

    scale = 1.0 / np.sqrt(DK)
    mband = (np.arange(7 * P)[None, :] - 3 * P
             >= np.arange(P)[:, None]).astype(np.float32)

    with_qk_bias = bool(np.any(b_qkv[:2 * D]))
    xts = [np.ascontiguousarray(x[b].T) for b in range(B)]
    in_maps = []
    for c in range(8):
        b, g = c // 2, c % 2
        sl = slice(g * GW, (g + 1) * GW)
        m = {
            "xt": xts[b],
            "wq": np.ascontiguousarray(W_qkv[:, g * GW:(g + 1) * GW]) * scale,
            "wk": np.ascontiguousarray(W_qkv[:, D + g * GW:D + (g + 1) * GW]),
            "wv": np.ascontiguousarray(W_qkv[:, 2 * D + g * GW:2 * D + (g + 1) * GW]),
            "wo": np.ascontiguousarray(W_out[sl, :]),
            "mband": mband,
        }
        if with_qk_bias:
            m["bq"] = np.ascontiguousarray(
                b_qkv[g * GW:(g + 1) * GW].reshape(NPAIR, P).T) * scale
            m["bk"] = np.ascontiguousarray(
                b_qkv[D + g * GW:D + (g + 1) * GW].reshape(NPAIR, P).T)
        in_maps.append(m)
    return in_maps, with_qk_bias


def kernel(x, W_qkv, b_qkv, W_out, b_out):
    in_maps, with_qk_bias = _prep_inputs(x, W_qkv, b_qkv, W_out)

    key = ("nc", with_qk_bias)
    if key not in _NC_CACHE:
        _NC_CACHE[key] = build_nc(with_qk_bias)
    nc = _NC_CACHE[key]

    res = bass_utils.run_bass_kernel_spmd(nc, in_maps, core_ids=list(range(8)))
    parts = [r["y"] for r in res.results]

    b_qkv = np.asarray(b_qkv, dtype=np.float32)
    W_out_np = np.asarray(W_out, dtype=np.float32)
    # v-bias passes through attention (rows of attn sum to 1) and out-proj is
    # linear: contribution = b_v @ W_out; b_out adds directly.
    corr = (b_qkv[2 * D:3 * D] @ W_out_np
            + np.asarray(b_out, dtype=np.float32)).astype(np.float32)

    out = np.empty((B, L, D), dtype=np.float32)
    for b in range(B):
        out[b] = parts[2 * b] + parts[2 * b + 1] + corr
    return out
